# revision 44
# baseline (speedup 1.0000x reference)
"""AnyPrecisionLinear (4-bit LUT dequant + CSR outliers + bias) on 8 TRN2 cores.

Sharding: 8-way over out_features (O); tokens replicated.
Core c handles o in [512*c, 512*(c+1)), all 8192 tokens.

Device does all value math:
  - W rows built from lut via GPSIMD local_scatter of full LUT values
    (host precomputes pure index slot tables from qweight bits).
  - CSR outlier values: lut part selected by one tiny local_scatter from the
    replicated-lut pattern, added to DMA'd CSR values on DVE, merged into the
    same per-chunk scatter.
  - W transposed on the PE (is_transpose matmul), 4 blocks per PSUM tile.
  - GEMM on TensorE (bf16, f32 PSUM accum) in two phases over i-blocks so
    it starts after only the first quarter of the dequant; phase 2 adds its
    partial into y in DRAM (gpsimd accum DMA). Chunk pairs interleave on two
    PSUM banks to hide the PE array drain. Bias added on DVE at copy-out
    from a partition_broadcast bias row.
Host does only layout/index work: sharding, bit-plane->index repack, slot
tables, CSR indptr parsing + dedup, x transpose to [chunk, i, token] layout
(+ f32->bf16 rounding), output concat.
"""

import numpy as np
from contextlib import ExitStack

import ml_dtypes

# Problem constants (hardcoded per harness contract).
B, S, I, O = 4, 2048, 4096, 4096
W_BITS = 4
NT = B * S                # 8192 tokens
N_CORES = 8
O_SH = O // N_CORES       # 512 out features per core
OT = O_SH // 128          # 4 o-tiles of 128 rows per core
NCHUNK = NT // 128        # 64 token chunks
IC = I // 128             # 32 i-blocks
CH = 1024                 # i-chunk size for local_scatter
NCH = I // CH             # 4

XB = 2                    # x chunks per DMA
YB = 4                    # y chunks per DMA store

_GRAPH_CACHE = {}

_EYE = np.eye(128, dtype=ml_dtypes.bfloat16)


def _host_indices(qweight):
    """bit-planes -> 4-bit index array [O, I] (uint8). Pure bit relayout."""
    shifts = np.arange(32, dtype=np.int32)
    bits = ((qweight[:, :, :, None] >> shifts) & 1).astype(np.uint8)
    planew = (1 << (W_BITS - 1 - np.arange(W_BITS))).astype(np.uint8)
    idx = (bits * planew[:, None, None, None]).sum(axis=0, dtype=np.int32)
    return idx.reshape(O, I).astype(np.uint8)


def _scatter_tables(idx, rows, cols, vals):
    """Slot tables for the merged dequant+CSR local_scatter.

    Per o-row the device holds one data strip [4*CSE + NI]:
      [0 : 4*CSE)        comb slots: chunk-major CSR values (cv + lut[v])
      [4*CSE : 4*CSE+NI) pattern slots: slot 16*s+v holds lut[o, v]
    Chunk ch's scatter uses idx table tbl[o, ch] over the whole strip; slots
    belonging to other chunks (or unused) are -1.

    Returns:
      tbl   [O, NCH, W] int16  scatter dest (position in chunk) or -1
      cst   [O, SC16]   int16  tiny-scatter dest (comb slot) for CSR lut part
      cva   [O, NCH*CSE] f32   CSR values (0 pad)
      CSE, NI, SC16
    """
    nnz = cols.shape[0]
    row_ids = (np.searchsorted(rows, np.arange(nnz), side="right") - 1).astype(np.int64)
    key = row_ids * I + cols.astype(np.int64)
    uk, inv = np.unique(key, return_inverse=True)
    v2 = np.zeros(len(uk), np.float64)
    np.add.at(v2, inv, vals.astype(np.float64))
    r2 = uk // I
    c2 = uk % I
    ch2 = (c2 // CH).astype(np.int64)
    cl2 = (c2 % CH).astype(np.int16)

    grp = r2 * NCH + ch2                       # ascending (uk sorted)
    _, gstart, gcount = np.unique(grp, return_index=True, return_counts=True)
    CSE = int(gcount.max())
    CSE += CSE % 2
    CSE = max(CSE, 2)
    rank = np.arange(len(uk)) - np.repeat(gstart, gcount)

    is_csr = np.zeros((O, NCH, CH), bool)
    is_csr[r2, ch2, cl2] = True

    # ---- dequant slots: all 16 values, csr positions excluded ----
    idx4 = idx.reshape(O, NCH, CH).astype(np.int16)
    idxm = np.where(is_csr, np.int16(16), idx4)          # sentinel sorts last
    order = np.argsort(idxm, axis=-1, kind="stable").astype(np.int16)
    sortedv = np.take_along_axis(idxm, order.astype(np.int64), axis=-1)
    keep = sortedv < 16
    cnt = np.zeros((O, NCH, 16), np.int32)
    for v in range(16):
        cnt[:, :, v] = (idxm == v).sum(-1)
    Smax = int(cnt.max())
    NI = 16 * Smax
    cstart = np.concatenate(
        [np.zeros((O, NCH, 1), np.int32), np.cumsum(cnt, -1)[:, :, :-1]], -1
    )
    srank = np.arange(CH)[None, None, :] - np.take_along_axis(
        cstart, np.minimum(sortedv, 15).astype(np.int64), axis=-1
    )
    W = 4 * CSE + NI
    tbl = np.full((O, NCH, W + 2), -1, np.int16)
    slot = (4 * CSE + 16 * srank + sortedv).astype(np.int64)
    np.put_along_axis(
        tbl, np.where(keep, slot, W + 1),
        np.where(keep, order, -1), axis=-1,
    )
    tbl = tbl[:, :, :W].copy()

    # ---- csr dest slots in the per-chunk tables ----
    comb_slot = (ch2 * CSE + rank).astype(np.int64)
    tbl[r2, ch2, comb_slot] = cl2

    # ---- csr values + tiny-scatter table (lut part of comb) ----
    cva = np.zeros((O, NCH * CSE), np.float32)
    cva[r2, comb_slot] = v2.astype(np.float32)
    vsl = idx4[r2, ch2, cl2.astype(np.int64)]            # lut index per entry
    # occurrence rank of (row, v) among csr entries of that row
    keyrv = r2 * 16 + vsl
    ord2 = np.argsort(keyrv, kind="stable")
    kr_sorted = keyrv[ord2]
    _, g2start, g2count = np.unique(kr_sorted, return_index=True, return_counts=True)
    rank2 = np.empty(len(uk), np.int64)
    rank2[ord2] = np.arange(len(uk)) - np.repeat(g2start, g2count)
    SC = max(int(g2count.max()), 1)
    SC16 = 16 * SC
    cst = np.full((O, SC16), -1, np.int16)
    cst[r2, 16 * rank2 + vsl] = comb_slot.astype(np.int16)
    return tbl, cst, cva, CSE, NI, SC16


def _build_graph(CSE, NI, SC16):
    import concourse.bass as bass
    import concourse.bacc as bacc
    import concourse.tile as tile
    from concourse import mybir

    dt = mybir.dt
    nc = bacc.Bacc("TRN2", target_bir_lowering=False, debug=False)

    WCOL = 4 * CSE + NI
    x_d = nc.dram_tensor("x", [NCHUNK, 128, I], dt.bfloat16, kind="ExternalInput")
    lut_d = nc.dram_tensor("lut", [OT, 128, 16], dt.float32, kind="ExternalInput")
    qid_d = nc.dram_tensor("qidx", [NCH, OT, 128, WCOL], dt.int16, kind="ExternalInput")
    cst_d = nc.dram_tensor("cst", [OT, 128, SC16], dt.int16, kind="ExternalInput")
    cva_d = nc.dram_tensor("cvals", [OT, 128, NCH * CSE], dt.float32, kind="ExternalInput")
    bias_d = nc.dram_tensor("bias", [1, O_SH], dt.float32, kind="ExternalInput")
    eye_d = nc.dram_tensor("eye", [128, 128], dt.bfloat16, kind="ExternalInput")
    y_d = nc.dram_tensor("y", [NCHUNK, 128, O_SH], dt.float32, kind="ExternalOutput")

    # GEMM phases over i-block ranges; phase k covers scatter-chunk chs PH[k].
    # Matmuls of chunk PAIRS interleave on two PSUM banks to hide the PE
    # array drain (serial fill->drain on one bank costs ~46ns/matmul).
    # phases: (ic0, ic1, ocol0, ocol1, accum) over i-blocks; phase 0 covers
    # scatter-chunk ch0, phase 1 the rest (accumulated into DRAM f32).
    PH = [
        (0, 8, 0, 512, False),
        (8, 32, 0, 512, True),
    ]
    # transpose-round insertions: (phase, pair-index) -> (ch, tile_lo, tile_hi)
    TINS = {
        (0, 10): (1, 0, 4),
        (0, 19): (2, 0, 4),
        (0, 28): (3, 0, 4),
    }

    with tile.TileContext(nc) as tc, ExitStack() as ctx:
        const = ctx.enter_context(tc.tile_pool(name="const", bufs=1))
        dpool = ctx.enter_context(tc.tile_pool(name="dp", bufs=1))
        qpool = ctx.enter_context(tc.tile_pool(name="qp", bufs=3))
        spool = ctx.enter_context(tc.tile_pool(name="sp", bufs=2))
        wpool = ctx.enter_context(tc.tile_pool(name="w", bufs=12))
        xpool = ctx.enter_context(tc.tile_pool(name="x", bufs=3))
        ypool = ctx.enter_context(tc.tile_pool(name="ya", bufs=2))
        y2pool = ctx.enter_context(tc.tile_pool(name="yb", bufs=2))
        psum = ctx.enter_context(
            tc.tile_pool(name="ps", bufs=4, space=bass.MemorySpace.PSUM)
        )
        pst = ctx.enter_context(
            tc.tile_pool(name="pst", bufs=2, space=bass.MemorySpace.PSUM)
        )

        # Resident transposed weights: WT[p, 512*ic + 128*t + ol] = W[128*t+ol, 128*ic+p]
        WT = const.tile([128, IC * O_SH], dt.bfloat16)

        eye = const.tile([128, 128], dt.bfloat16)
        nc.scalar.dma_start(eye[:, :], eye_d[:, :])

        # ---- per-tile preps: pattern + CSR comb values ----
        datas, wqs = [], []

        def qround_load(ch, dma_eng):
            qr = qpool.tile([128, OT, WCOL], dt.int16, tag="qr")
            dma_eng.dma_start(qr[:, :, :], qid_d[ch].rearrange("t p w -> p t w"))
            return qr

        def scatter_round(ch, qr):
            for t in range(OT):
                wq = wpool.tile([128, CH], dt.bfloat16, tag="wq")
                nc.gpsimd.local_scatter(
                    wq[:, :], datas[t][:, :], qr[:, t, :],
                    channels=128, num_elems=CH, num_idxs=WCOL,
                )
                wqs.append(wq)

        for t in range(OT):
            lutf = spool.tile([128, 16], dt.float32, tag="lutf")
            nc.sync.dma_start(lutf[:, :], lut_d[t])
            data = dpool.tile([128, WCOL], dt.bfloat16, tag=f"data{t}")
            nc.vector.tensor_copy(data[:, 4 * CSE : 4 * CSE + 16], lutf[:, :])
            sz = 16
            while sz < NI:
                cp = min(sz, NI - sz)
                nc.vector.tensor_copy(
                    data[:, 4 * CSE + sz : 4 * CSE + sz + cp],
                    data[:, 4 * CSE : 4 * CSE + cp],
                )
                sz += cp
            cstt = spool.tile([128, SC16], dt.int16, tag="cst")
            nc.sync.dma_start(cstt[:, :], cst_d[t])
            nc.gpsimd.local_scatter(
                data[:, 0 : 4 * CSE], data[:, 4 * CSE : 4 * CSE + SC16],
                cstt[:, :], channels=128, num_elems=4 * CSE, num_idxs=SC16,
            )
            cvf = spool.tile([128, NCH * CSE], dt.float32, tag="cvf")
            nc.sync.dma_start(cvf[:, :], cva_d[t])
            cvb = spool.tile([128, NCH * CSE], dt.bfloat16, tag="cvb")
            nc.vector.tensor_copy(cvb[:, :], cvf[:, :])
            nc.vector.tensor_add(
                data[:, 0 : 4 * CSE], data[:, 0 : 4 * CSE], cvb[:, :]
            )
            datas.append(data)
            if t == 0:
                # ch0 tables load right after tile 0's prep DMAs: its
                # transfer overlaps the remaining preps.
                qr0 = qround_load(0, nc.sync)

        browp = const.tile([1, O_SH], dt.float32)
        nc.scalar.dma_start(browp[:, :], bias_d[:, :])
        brow = const.tile([128, O_SH], dt.float32)
        nc.gpsimd.partition_broadcast(brow[:, :], browp[:, :])

        scatter_round(0, qr0)

        def transpose_round(ch, tlo=0, thi=OT):
            for t in range(tlo, thi):
                wq = wqs[4 * ch + t]
                for g in range(2):
                    pt = pst.tile([128, 512], dt.bfloat16, tag="pt")
                    for k in range(4):
                        l = 4 * g + k
                        nc.tensor.transpose(
                            pt[:, 128 * k : 128 * (k + 1)],
                            wq[:, 128 * l : 128 * (l + 1)],
                            eye[:, :],
                        )
                    ic0 = 8 * ch + 4 * g
                    dst = WT[:, :].rearrange("p (ic o) -> p ic o", o=O_SH)[
                        :, ic0 : ic0 + 4, 128 * t : 128 * (t + 1)
                    ]
                    nc.vector.tensor_copy(
                        dst, pt[:, :].rearrange("p (a b) -> p a b", b=128)
                    )

        # ch1-3 table round-loads go on the ACT queue (3 quick DMAs, done
        # before the y stores start).
        for ch in range(1, NCH):
            qr = qround_load(ch, nc.scalar)
            scatter_round(ch, qr)
        transpose_round(0)

        # ---- GEMM: phases over (i-block, o-column) tiles; chunk pairs
        # interleave on two PSUM banks ----
        for ph, (ica, icb, oc0, oc1, accum) in enumerate(PH):
            c0, c1 = 128 * ica, 128 * icb
            ocw = oc1 - oc0
            for p in range(NCHUNK // 2):
                n0 = 2 * p
                xT = xpool.tile([128, 2, c1 - c0], dt.bfloat16, tag="xT")
                nc.sync.dma_start(
                    xT[:, :, :],
                    x_d[n0 : n0 + 2][:, :, c0:c1].rearrange("a b c -> b a c"),
                )
                if n0 % YB == 0:
                    pool = y2pool if accum else ypool
                    yo = pool.tile([128, YB, ocw], dt.float32, tag="yo")
                ps0 = psum.tile([128, O_SH], dt.float32, tag="ps")
                ps1 = psum.tile([128, O_SH], dt.float32, tag="ps")
                pss = [ps0, ps1]
                for ic in range(ica, icb):
                    for j in range(2):
                        nc.tensor.matmul(
                            pss[j][:, 0:ocw],
                            xT[:, j, 128 * ic - c0 : 128 * (ic + 1) - c0],
                            WT[:, O_SH * ic + oc0 : O_SH * ic + oc1],
                            start=(ic == ica), stop=(ic == icb - 1),
                        )
                for j in range(2):
                    if accum:
                        nc.vector.tensor_copy(
                            yo[:, (n0 + j) % YB, :], pss[j][:, 0:ocw]
                        )
                    else:
                        nc.vector.tensor_add(
                            yo[:, (n0 + j) % YB, :], pss[j][:, 0:ocw],
                            brow[:, oc0:oc1],
                        )
                if accum and n0 + 4 >= NCHUNK:
                    # split the final batch into 2-chunk stores to trim the tail
                    half = 0 if n0 + 4 == NCHUNK else 1
                    nc.gpsimd.dma_start(
                        y_d[n0 : n0 + 2].rearrange("a b c -> b a c"),
                        yo[:, 2 * half : 2 * half + 2, :],
                        accum_op=mybir.AluOpType.add,
                    )
                elif (n0 + 1) % YB == YB - 1:
                    ysl = y_d[n0 + 2 - YB : n0 + 2][:, :, oc0:oc1].rearrange(
                        "a b c -> b a c"
                    )
                    if accum:
                        nc.gpsimd.dma_start(
                            ysl, yo[:, :, :], accum_op=mybir.AluOpType.add
                        )
                    else:
                        nc.scalar.dma_start(ysl, yo[:, :, :])
                ins = TINS.get((ph, p))
                if ins is not None:
                    transpose_round(*ins)

    nc.compile()
    return nc


def _prep_inputs(x, qweight, lut, rows, cols, vals, bias):
    x = np.asarray(x, dtype=np.float32)
    qweight = np.asarray(qweight, dtype=np.int32)
    lut = np.asarray(lut, dtype=np.float32)
    rows = np.asarray(rows, dtype=np.int64)
    cols = np.asarray(cols, dtype=np.int64)
    vals = np.asarray(vals, dtype=np.float32)
    bias = np.asarray(bias, dtype=np.float32)

    idx = _host_indices(qweight)
    tbl, cst, cva, CSE, NI, SC16 = _scatter_tables(idx, rows, cols, vals)

    # x -> [chunk, i, token] bf16 (pure relayout + dtype rounding)
    xt = np.ascontiguousarray(
        x.reshape(NCHUNK, 128, IC, 128).transpose(0, 3, 2, 1)
    ).reshape(NCHUNK, 128, I).astype(ml_dtypes.bfloat16)

    in_maps = []
    for c in range(N_CORES):
        osl = slice(O_SH * c, O_SH * (c + 1))
        in_maps.append(
            {
                "x": xt,
                "lut": np.ascontiguousarray(lut[osl].reshape(OT, 128, 16)),
                "qidx": np.ascontiguousarray(
                    tbl[osl].reshape(OT, 128, NCH, -1).transpose(2, 0, 1, 3)
                ),
                "cst": np.ascontiguousarray(cst[osl].reshape(OT, 128, SC16)),
                "cvals": np.ascontiguousarray(cva[osl].reshape(OT, 128, -1)),
                "bias": np.ascontiguousarray(bias[osl].reshape(1, O_SH)),
                "eye": _EYE,
            }
        )
    return in_maps, CSE, NI, SC16


def _run(inputs, trace=False, trace_kwargs=None):
    from concourse.bass_utils import run_bass_kernel_spmd

    in_maps, CSE, NI, SC16 = _prep_inputs(**inputs)

    key = (CSE, NI, SC16)
    if key not in _GRAPH_CACHE:
        _GRAPH_CACHE[key] = _build_graph(CSE, NI, SC16)
    nc = _GRAPH_CACHE[key]

    res = run_bass_kernel_spmd(
        nc, in_maps, core_ids=list(range(N_CORES)),
        trace=trace, **(trace_kwargs or {}),
    )
    out = np.empty((NT, O), np.float32)
    for c in range(N_CORES):
        yc = res.results[c]["y"].reshape(NT, O_SH)
        out[:, O_SH * c : O_SH * (c + 1)] = yc
    return out.reshape(B, S, O), res


def kernel(x, qweight, lut, rows, cols, vals, bias):
    out, _ = _run(dict(x=x, qweight=qweight, lut=lut, rows=rows,
                       cols=cols, vals=vals, bias=bias))
    return out


# revision 45
# speedup vs baseline: 1.1867x; 1.1867x over previous
"""AnyPrecisionLinear (4-bit LUT dequant + CSR outliers + bias) on 8 TRN2 cores.

Sharding: 8-way over out_features (O); tokens replicated.
Core c handles o in [512*c, 512*(c+1)), all 8192 tokens.

Device does all value math:
  - W rows built from lut via GPSIMD local_scatter of full LUT values
    (host precomputes pure index slot tables from qweight bits).
  - CSR outlier values: lut part selected by one tiny local_scatter from the
    replicated-lut pattern, added to DMA'd CSR values on DVE, merged into the
    same per-chunk scatter.
  - W transposed on the PE (is_transpose matmul), 4 blocks per PSUM tile.
  - GEMM on TensorE (bf16, f32 PSUM accum) in two phases over i-blocks so
    it starts after only the first quarter of the dequant; phase 2 adds its
    partial into y in DRAM (gpsimd accum DMA). Chunk pairs interleave on two
    PSUM banks to hide the PE array drain. Bias added on DVE at copy-out
    from a partition_broadcast bias row.
Host does only layout/index work: sharding, bit-plane->index repack, slot
tables, CSR indptr parsing + dedup, x transpose to [chunk, i, token] layout
(+ f32->bf16 rounding), output concat.
"""

import numpy as np
from contextlib import ExitStack

import ml_dtypes

# Problem constants (hardcoded per harness contract).
B, S, I, O = 4, 2048, 4096, 4096
W_BITS = 4
NT = B * S                # 8192 tokens
N_CORES = 8
O_SH = O // N_CORES       # 512 out features per core
OT = O_SH // 128          # 4 o-tiles of 128 rows per core
NCHUNK = NT // 128        # 64 token chunks
IC = I // 128             # 32 i-blocks
CH = 1024                 # i-chunk size for local_scatter
NCH = I // CH             # 4

XB = 2                    # x chunks per DMA
YB = 4                    # y chunks per DMA store

_GRAPH_CACHE = {}

_EYE = np.eye(128, dtype=ml_dtypes.bfloat16)


def _host_indices(qweight):
    """bit-planes -> 4-bit index array [O, I] (uint8). Pure bit relayout."""
    shifts = np.arange(32, dtype=np.int32)
    bits = ((qweight[:, :, :, None] >> shifts) & 1).astype(np.uint8)
    planew = (1 << (W_BITS - 1 - np.arange(W_BITS))).astype(np.uint8)
    idx = (bits * planew[:, None, None, None]).sum(axis=0, dtype=np.int32)
    return idx.reshape(O, I).astype(np.uint8)


def _scatter_tables(idx, rows, cols, vals):
    """Slot tables for the merged dequant+CSR local_scatter.

    Per o-row the device holds one data strip [4*CSE + NI]:
      [0 : 4*CSE)        comb slots: chunk-major CSR values (cv + lut[v])
      [4*CSE : 4*CSE+NI) pattern slots: slot 16*s+v holds lut[o, v]
    Chunk ch's scatter uses idx table tbl[o, ch] over the whole strip; slots
    belonging to other chunks (or unused) are -1.

    Returns:
      tbl   [O, NCH, W] int16  scatter dest (position in chunk) or -1
      cst   [O, SC16]   int16  tiny-scatter dest (comb slot) for CSR lut part
      cva   [O, NCH*CSE] f32   CSR values (0 pad)
      CSE, NI, SC16
    """
    nnz = cols.shape[0]
    row_ids = (np.searchsorted(rows, np.arange(nnz), side="right") - 1).astype(np.int64)
    key = row_ids * I + cols.astype(np.int64)
    uk, inv = np.unique(key, return_inverse=True)
    v2 = np.zeros(len(uk), np.float64)
    np.add.at(v2, inv, vals.astype(np.float64))
    r2 = uk // I
    c2 = uk % I
    ch2 = (c2 // CH).astype(np.int64)
    cl2 = (c2 % CH).astype(np.int16)

    is_csr = np.zeros((O, NCH, CH), bool)
    is_csr[r2, ch2, cl2] = True

    # ---- dequant slots: all 16 values, csr positions excluded ----
    idx4 = idx.reshape(O, NCH, CH).astype(np.int16)
    idxm = np.where(is_csr, np.int16(16), idx4)          # sentinel sorts last
    order = np.argsort(idxm, axis=-1, kind="stable").astype(np.int16)
    sortedv = np.take_along_axis(idxm, order.astype(np.int64), axis=-1)
    keep = sortedv < 16
    cnt = np.zeros((O, NCH, 16), np.int32)
    for v in range(16):
        cnt[:, :, v] = (idxm == v).sum(-1)
    # Cap the pattern grid: occurrences beyond S0 of a value in a chunk are
    # routed through the comb path as zero-valued outliers (0 + lut[o,v]).
    S0 = min(int(cnt.max()), 88)
    NI = 16 * S0
    cstart = np.concatenate(
        [np.zeros((O, NCH, 1), np.int32), np.cumsum(cnt, -1)[:, :, :-1]], -1
    )
    srank = np.arange(CH)[None, None, :] - np.take_along_axis(
        cstart, np.minimum(sortedv, 15).astype(np.int64), axis=-1
    )
    keep_pat = keep & (srank < S0)
    ovf = keep & (srank >= S0)

    # ---- combined comb entries: CSR + pattern overflow ----
    oo, cc, jj = np.nonzero(ovf)
    ro = np.concatenate([r2, oo])
    cho = np.concatenate([ch2, cc])
    clo = np.concatenate(
        [cl2.astype(np.int64), order[oo, cc, jj].astype(np.int64)]
    )
    cvv = np.concatenate([v2, np.zeros(len(oo))])
    vso = np.concatenate(
        [idx4[r2, ch2, cl2.astype(np.int64)], sortedv[oo, cc, jj]]
    ).astype(np.int64)

    key2 = ro * NCH + cho
    ord3 = np.argsort(key2, kind="stable")
    ro, cho, clo, cvv, vso = (a[ord3] for a in (ro, cho, clo, cvv, vso))
    _, gstart, gcount = np.unique(ro * NCH + cho, return_index=True,
                                  return_counts=True)
    CSE = int(gcount.max())
    CSE += CSE % 2
    CSE = max(CSE, 2)
    rank = np.arange(len(ro)) - np.repeat(gstart, gcount)

    W = 4 * CSE + NI
    tbl = np.full((O, NCH, W + 2), -1, np.int16)
    slot = (4 * CSE + 16 * srank + sortedv).astype(np.int64)
    np.put_along_axis(
        tbl, np.where(keep_pat, slot, W + 1),
        np.where(keep_pat, order, -1), axis=-1,
    )
    tbl = tbl[:, :, :W].copy()

    # ---- comb dest slots in the per-chunk tables ----
    comb_slot = (cho * CSE + rank).astype(np.int64)
    tbl[ro, cho, comb_slot] = clo.astype(np.int16)

    # ---- comb values + tiny-scatter table (lut part of comb) ----
    cva = np.zeros((O, NCH * CSE), np.float32)
    cva[ro, comb_slot] = cvv.astype(np.float32)
    # occurrence rank of (row, v) among comb entries of that row
    keyrv = ro * 16 + vso
    ord2 = np.argsort(keyrv, kind="stable")
    kr_sorted = keyrv[ord2]
    _, g2start, g2count = np.unique(kr_sorted, return_index=True, return_counts=True)
    rank2 = np.empty(len(ro), np.int64)
    rank2[ord2] = np.arange(len(ro)) - np.repeat(g2start, g2count)
    SC = max(int(g2count.max()), 1)
    SC16 = 16 * SC
    assert SC16 <= NI, (SC16, NI)
    cst = np.full((O, SC16), -1, np.int16)
    cst[ro, 16 * rank2 + vso] = comb_slot.astype(np.int16)
    return tbl, cst, cva, CSE, NI, SC16


def _build_graph(CSE, NI, SC16):
    import concourse.bass as bass
    import concourse.bacc as bacc
    import concourse.tile as tile
    from concourse import mybir

    dt = mybir.dt
    nc = bacc.Bacc("TRN2", target_bir_lowering=False, debug=False)

    WCOL = 4 * CSE + NI
    x_d = nc.dram_tensor("x", [NCHUNK, 128, I], dt.bfloat16, kind="ExternalInput")
    lut_d = nc.dram_tensor("lut", [OT, 128, 16], dt.float32, kind="ExternalInput")
    qid_d = nc.dram_tensor("qidx", [NCH, OT, 128, WCOL], dt.int16, kind="ExternalInput")
    cst_d = nc.dram_tensor("cst", [OT, 128, SC16], dt.int16, kind="ExternalInput")
    cva_d = nc.dram_tensor("cvals", [OT, 128, NCH * CSE], dt.float32, kind="ExternalInput")
    bias_d = nc.dram_tensor("bias", [1, O_SH], dt.float32, kind="ExternalInput")
    eye_d = nc.dram_tensor("eye", [128, 128], dt.bfloat16, kind="ExternalInput")
    y_d = nc.dram_tensor("y", [NCHUNK, 128, O_SH], dt.float32, kind="ExternalOutput")

    # GEMM phases over i-block ranges; phase k covers scatter-chunk chs PH[k].
    # Matmuls of chunk PAIRS interleave on two PSUM banks to hide the PE
    # array drain (serial fill->drain on one bank costs ~46ns/matmul).
    # phases: (ic0, ic1, ocol0, ocol1, accum) over i-blocks; phase 0 covers
    # scatter-chunk ch0, phase 1 the rest (accumulated into DRAM f32).
    PH = [
        (0, 8, 0, 512, False),
        (8, 32, 0, 512, True),
    ]
    # transpose-round insertions: (phase, pair-index) -> (ch, tile_lo, tile_hi)
    TINS = {
        (0, 10): (1, 0, 4),
        (0, 19): (2, 0, 4),
        (0, 28): (3, 0, 4),
    }

    with tile.TileContext(nc) as tc, ExitStack() as ctx:
        const = ctx.enter_context(tc.tile_pool(name="const", bufs=1))
        dpool = ctx.enter_context(tc.tile_pool(name="dp", bufs=1))
        qpool = ctx.enter_context(tc.tile_pool(name="qp", bufs=3))
        spool = ctx.enter_context(tc.tile_pool(name="sp", bufs=2))
        wpool = ctx.enter_context(tc.tile_pool(name="w", bufs=12))
        xpool = ctx.enter_context(tc.tile_pool(name="x", bufs=3))
        ypool = ctx.enter_context(tc.tile_pool(name="ya", bufs=2))
        y2pool = ctx.enter_context(tc.tile_pool(name="yb", bufs=2))
        psum = ctx.enter_context(
            tc.tile_pool(name="ps", bufs=4, space=bass.MemorySpace.PSUM)
        )
        pst = ctx.enter_context(
            tc.tile_pool(name="pst", bufs=2, space=bass.MemorySpace.PSUM)
        )

        # Resident transposed weights: WT[p, 512*ic + 128*t + ol] = W[128*t+ol, 128*ic+p]
        WT = const.tile([128, IC * O_SH], dt.bfloat16)

        eye = const.tile([128, 128], dt.bfloat16)
        nc.scalar.dma_start(eye[:, :], eye_d[:, :])

        # ---- per-tile preps: pattern + CSR comb values ----
        datas, wqs = [], []

        def qround_load(ch, dma_eng):
            qr = qpool.tile([128, OT, WCOL], dt.int16, tag="qr")
            dma_eng.dma_start(qr[:, :, :], qid_d[ch].rearrange("t p w -> p t w"))
            return qr

        def scatter_round(ch, qr):
            for t in range(OT):
                wq = wpool.tile([128, CH], dt.bfloat16, tag="wq")
                nc.gpsimd.local_scatter(
                    wq[:, :], datas[t][:, :], qr[:, t, :],
                    channels=128, num_elems=CH, num_idxs=WCOL,
                )
                wqs.append(wq)

        for t in range(OT):
            lutf = spool.tile([128, 16], dt.float32, tag="lutf")
            nc.sync.dma_start(lutf[:, :], lut_d[t])
            data = dpool.tile([128, WCOL], dt.bfloat16, tag=f"data{t}")
            nc.vector.tensor_copy(data[:, 4 * CSE : 4 * CSE + 16], lutf[:, :])
            sz = 16
            while sz < NI:
                cp = min(sz, NI - sz)
                nc.vector.tensor_copy(
                    data[:, 4 * CSE + sz : 4 * CSE + sz + cp],
                    data[:, 4 * CSE : 4 * CSE + cp],
                )
                sz += cp
            cstt = spool.tile([128, SC16], dt.int16, tag="cst")
            nc.sync.dma_start(cstt[:, :], cst_d[t])
            nc.gpsimd.local_scatter(
                data[:, 0 : 4 * CSE], data[:, 4 * CSE : 4 * CSE + SC16],
                cstt[:, :], channels=128, num_elems=4 * CSE, num_idxs=SC16,
            )
            cvf = spool.tile([128, NCH * CSE], dt.float32, tag="cvf")
            nc.sync.dma_start(cvf[:, :], cva_d[t])
            cvb = spool.tile([128, NCH * CSE], dt.bfloat16, tag="cvb")
            nc.vector.tensor_copy(cvb[:, :], cvf[:, :])
            nc.vector.tensor_add(
                data[:, 0 : 4 * CSE], data[:, 0 : 4 * CSE], cvb[:, :]
            )
            datas.append(data)
            if t == 0:
                # ch0 tables load right after tile 0's prep DMAs: its
                # transfer overlaps the remaining preps.
                qr0 = qround_load(0, nc.sync)

        browp = const.tile([1, O_SH], dt.float32)
        nc.scalar.dma_start(browp[:, :], bias_d[:, :])
        brow = const.tile([128, O_SH], dt.float32)
        nc.gpsimd.partition_broadcast(brow[:, :], browp[:, :])

        scatter_round(0, qr0)

        def transpose_round(ch, tlo=0, thi=OT):
            for t in range(tlo, thi):
                wq = wqs[4 * ch + t]
                for g in range(2):
                    pt = pst.tile([128, 512], dt.bfloat16, tag="pt")
                    for k in range(4):
                        l = 4 * g + k
                        nc.tensor.transpose(
                            pt[:, 128 * k : 128 * (k + 1)],
                            wq[:, 128 * l : 128 * (l + 1)],
                            eye[:, :],
                        )
                    ic0 = 8 * ch + 4 * g
                    dst = WT[:, :].rearrange("p (ic o) -> p ic o", o=O_SH)[
                        :, ic0 : ic0 + 4, 128 * t : 128 * (t + 1)
                    ]
                    nc.vector.tensor_copy(
                        dst, pt[:, :].rearrange("p (a b) -> p a b", b=128)
                    )

        # ch1-3 table round-loads go on the ACT queue (3 quick DMAs, done
        # before the y stores start).
        for ch in range(1, NCH):
            qr = qround_load(ch, nc.scalar)
            scatter_round(ch, qr)
        transpose_round(0)

        # ---- GEMM: phases over (i-block, o-column) tiles; chunk pairs
        # interleave on two PSUM banks ----
        for ph, (ica, icb, oc0, oc1, accum) in enumerate(PH):
            c0, c1 = 128 * ica, 128 * icb
            ocw = oc1 - oc0
            for p in range(NCHUNK // 2):
                n0 = 2 * p
                xT = xpool.tile([128, 2, c1 - c0], dt.bfloat16, tag="xT")
                nc.sync.dma_start(
                    xT[:, :, :],
                    x_d[n0 : n0 + 2][:, :, c0:c1].rearrange("a b c -> b a c"),
                )
                if n0 % YB == 0:
                    pool = y2pool if accum else ypool
                    yo = pool.tile([128, YB, ocw], dt.float32, tag="yo")
                ps0 = psum.tile([128, O_SH], dt.float32, tag="ps")
                ps1 = psum.tile([128, O_SH], dt.float32, tag="ps")
                pss = [ps0, ps1]
                for ic in range(ica, icb):
                    for j in range(2):
                        nc.tensor.matmul(
                            pss[j][:, 0:ocw],
                            xT[:, j, 128 * ic - c0 : 128 * (ic + 1) - c0],
                            WT[:, O_SH * ic + oc0 : O_SH * ic + oc1],
                            start=(ic == ica), stop=(ic == icb - 1),
                        )
                for j in range(2):
                    if accum:
                        nc.vector.tensor_copy(
                            yo[:, (n0 + j) % YB, :], pss[j][:, 0:ocw]
                        )
                    else:
                        nc.vector.tensor_add(
                            yo[:, (n0 + j) % YB, :], pss[j][:, 0:ocw],
                            brow[:, oc0:oc1],
                        )
                if accum and n0 + 4 >= NCHUNK:
                    # split the final batch into 2-chunk stores to trim the tail
                    half = 0 if n0 + 4 == NCHUNK else 1
                    nc.gpsimd.dma_start(
                        y_d[n0 : n0 + 2].rearrange("a b c -> b a c"),
                        yo[:, 2 * half : 2 * half + 2, :],
                        accum_op=mybir.AluOpType.add,
                    )
                elif (n0 + 1) % YB == YB - 1:
                    ysl = y_d[n0 + 2 - YB : n0 + 2][:, :, oc0:oc1].rearrange(
                        "a b c -> b a c"
                    )
                    if accum:
                        nc.gpsimd.dma_start(
                            ysl, yo[:, :, :], accum_op=mybir.AluOpType.add
                        )
                    else:
                        nc.scalar.dma_start(ysl, yo[:, :, :])
                ins = TINS.get((ph, p))
                if ins is not None:
                    transpose_round(*ins)

    nc.compile()
    return nc


def _prep_inputs(x, qweight, lut, rows, cols, vals, bias):
    x = np.asarray(x, dtype=np.float32)
    qweight = np.asarray(qweight, dtype=np.int32)
    lut = np.asarray(lut, dtype=np.float32)
    rows = np.asarray(rows, dtype=np.int64)
    cols = np.asarray(cols, dtype=np.int64)
    vals = np.asarray(vals, dtype=np.float32)
    bias = np.asarray(bias, dtype=np.float32)

    idx = _host_indices(qweight)
    tbl, cst, cva, CSE, NI, SC16 = _scatter_tables(idx, rows, cols, vals)

    # x -> [chunk, i, token] bf16 (pure relayout + dtype rounding)
    xt = np.ascontiguousarray(
        x.reshape(NCHUNK, 128, IC, 128).transpose(0, 3, 2, 1)
    ).reshape(NCHUNK, 128, I).astype(ml_dtypes.bfloat16)

    in_maps = []
    for c in range(N_CORES):
        osl = slice(O_SH * c, O_SH * (c + 1))
        in_maps.append(
            {
                "x": xt,
                "lut": np.ascontiguousarray(lut[osl].reshape(OT, 128, 16)),
                "qidx": np.ascontiguousarray(
                    tbl[osl].reshape(OT, 128, NCH, -1).transpose(2, 0, 1, 3)
                ),
                "cst": np.ascontiguousarray(cst[osl].reshape(OT, 128, SC16)),
                "cvals": np.ascontiguousarray(cva[osl].reshape(OT, 128, -1)),
                "bias": np.ascontiguousarray(bias[osl].reshape(1, O_SH)),
                "eye": _EYE,
            }
        )
    return in_maps, CSE, NI, SC16


def _run(inputs, trace=False, trace_kwargs=None):
    from concourse.bass_utils import run_bass_kernel_spmd

    in_maps, CSE, NI, SC16 = _prep_inputs(**inputs)

    key = (CSE, NI, SC16)
    if key not in _GRAPH_CACHE:
        _GRAPH_CACHE[key] = _build_graph(CSE, NI, SC16)
    nc = _GRAPH_CACHE[key]

    res = run_bass_kernel_spmd(
        nc, in_maps, core_ids=list(range(N_CORES)),
        trace=trace, **(trace_kwargs or {}),
    )
    out = np.empty((NT, O), np.float32)
    for c in range(N_CORES):
        yc = res.results[c]["y"].reshape(NT, O_SH)
        out[:, O_SH * c : O_SH * (c + 1)] = yc
    return out.reshape(B, S, O), res


def kernel(x, qweight, lut, rows, cols, vals, bias):
    out, _ = _run(dict(x=x, qweight=qweight, lut=lut, rows=rows,
                       cols=cols, vals=vals, bias=bias))
    return out
